# revision 38
# baseline (speedup 1.0000x reference)
"""Trainium2 Bass kernel for nn_MultiHeadedAttention (B=2,S=2048,D=1024,H=16).

Sharding: tensor-parallel over heads — 2 heads per core x 8 cores.
Each core computes its 2 heads' attention and a partial output projection
(y_partial [B*S, D], bf16); the host sums the 8 partials and adds bo.

Device pipeline per core (all matmul operands bf16, fp32 PSUM accumulate):
  qT/kT = W @ xT (feature-major), v = x @ WvT (feature-major + PE transpose)
  S^T[k,q] = K @ Q^T (row-packed 2 heads, contraction dk=64)
  p = exp(S^T) * expbm   (expbm = exp(bias)*(1-mask), host-precomputed, bf16)
  OT = V^T @ p (ones-augmented V: row 64 = softmax denominator)
  OTn = OT * recip(d) -> y_partial = OTn^T @ WoT

Scheduling (the performance-critical part — TensorE must stream gap-free or
the HAM clock-gate drops it from 2.0 GHz to 1.2 GHz):
  - attention inner loop is software-pipelined: PV(i-1) is emitted after
    scores(i) so the exp+mul latency hides under the next score matmuls;
  - every projection / Wo-projection chain is woven INTO the attention
    loops as PE filler, placed between the two heads' score matmuls where
    it also covers exp(A)'s PSUM-slot latency (ps_s ring, bufs=2);
  - DRAM layouts are host-pre-tiled so every DMA is large + contiguous
    (x: 2 MiB per transfer, expbm: 512 KiB per transfer).
"""

import numpy as np
import ml_dtypes

import concourse.bass as bass
import concourse.tile as tile
from concourse import bacc, mybir
from concourse.bass import ts
from concourse.bass_utils import run_bass_kernel_spmd
from concourse.masks import make_identity

BF16 = ml_dtypes.bfloat16

B, S, D, H = 2, 2048, 1024, 16
N_CORES = 8
HC = H // N_CORES          # heads per core = 2
DK = D // H                # 64
DKC = HC * DK              # head dims per core = 128
P = 128
T = B * S                  # 4096 tokens
KO = D // P                # 8 feature k-subtiles
TC = 512                   # token chunk for projections
QC = 1024                  # q chunk for attention phase
NKS = S // P               # 16 k-subtiles per batch
NQC = S // QC              # 2 q-chunks per batch

bf = mybir.dt.bfloat16
f32 = mybir.dt.float32

VB = DK + 1                # vB column offset in v_sb
VW = DKC + 2               # v_sb row width


class _Ctx:
    pass


def _load_xpair(nc, g, p, fine=False):
    """Prefetch the double-width (1024-token) x tiles for chunk pair p.
    Host pre-tiles the layout so each DMA is one fully-contiguous 2 MiB
    block (16 KiB per partition). fine=True splits into per-ko 512 KiB
    sub-DMAs in consumption order so the first projection matmul can
    start ~1 us in instead of waiting for the full 6 MiB."""
    io = g.io
    for tag, src in (("xk", "xkT"), ("xv", "xvT"), ("xq", "xqT")):
        t = g.stream_pool.tile([P, KO, 2 * TC], bf, tag=tag, bufs=2,
                               name=f"{tag}w{p}")
        if fine:
            for ko in range(KO):
                nc.sync.dma_start(t[:, ko, :], io[src][p, :, ko, :])
        else:
            nc.sync.dma_start(t[:], io[src][p])
        g.xw[(tag, p)] = t


def _expbm_dma(nc, g, b, qc, kg):
    """Issue the expbm DMA for k-group kg (2 k-subtiles x QC per head) on
    the sync queue, one fully-contiguous 512 KiB transfer per head. Kept
    OFF the scalar queue: the exp ACTIVATEs pace the whole pipeline and
    must not sit behind DMA triggers."""
    io = g.io
    mA = g.work_pool.tile([P, 2, QC], bf, tag="mA", bufs=2,
                          name=f"mAg{b}_{qc}_{kg}")
    nc.sync.dma_start(mA[:], io["expbm"][b, 0, qc, kg])
    mB = g.work_pool.tile([P, 2, QC], bf, tag="mB", bufs=2,
                          name=f"mBg{b}_{qc}_{kg}")
    nc.sync.dma_start(mB[:], io["expbm"][b, 1, qc, kg])
    g.mgrp[(b, qc, kg)] = (mA, mB)


def _proj_chains(nc, g, c):
    """Build [k, v, q] emission closures for projection chunk c (512 tokens).
    Each closure is ~1.7-2.5us of dense PE work, used as PE filler inside
    attention chunks so the TensorE stream never gaps (keeps HAM at 8/8)."""
    h = c % 2
    p = c // 2

    def k_chain():
        ps_k = g.psum_pool.tile([P, QC], f32, tag="ps_s", bufs=2, name=f"psk{c}")
        for ko in range(KO):
            nc.tensor.matmul(
                ps_k[:, :TC], g.wk_sb[:, ko, :],
                g.xw[("xk", p)][:, ko, ts(h, TC)],
                start=(ko == 0), stop=(ko == KO - 1),
            )
        nc.vector.tensor_copy(g.kT_sb[:, ts(c, TC)], ps_k[:, :TC])

    def v_chain():
        # v computed feature-major (dense N=512 matmuls), then moved into
        # the token-major layout PV needs via the DMA transpose engine —
        # zero TensorE / VectorE cost beyond the PSUM->SBUF cast.
        ps_vT = g.psum_pool.tile([P, QC], f32, tag="ps_s", bufs=2,
                                 name=f"psvT{c}")
        for ko in range(KO):
            nc.tensor.matmul(
                ps_vT[:, :TC], g.wv_sb[:, ko, :],
                g.xw[("xv", p)][:, ko, ts(h, TC)],
                start=(ko == 0), stop=(ko == KO - 1),
            )
        vt_tmp = g.work_pool.tile([P, TC], bf, tag="vtt", bufs=2, name=f"vtt{c}")
        nc.vector.tensor_copy(vt_tmp[:], ps_vT[:, :TC])
        for tt in range(TC // P):
            ps_t = g.psum_pool.tile([P, P], bf, tag="ps_s", bufs=2,
                                    name=f"pst{c}_{tt}")
            nc.tensor.transpose(ps_t[:], vt_tmp[:, ts(tt, P)], g.ident_sb[:])
            vt_i = c * (TC // P) + tt
            nc.vector.tensor_copy(g.v_sb[:, vt_i, 0:DK], ps_t[:, 0:DK])
            nc.vector.tensor_copy(g.v_sb[:, vt_i, VB:VB + DK], ps_t[:, DK:DKC])

    def q_chain():
        ps_q = g.psum_pool.tile([P, QC], f32, tag="ps_s", bufs=2, name=f"psq{c}")
        for ko in range(KO):
            nc.tensor.matmul(
                ps_q[:, :TC], g.wq_sb[:, ko, :],
                g.xw[("xq", p)][:, ko, ts(h, TC)],
                start=(ko == 0), stop=(ko == KO - 1),
            )
        nc.vector.tensor_copy(g.qT_sb[:, ts(c, TC)], ps_q[:, :TC])

    return [k_chain, v_chain, q_chain]


def _attention_chunk(nc, g, b, qc, fillers=(), pending=None, prefetch=None,
                     tail_factory=None):
    """Emit attention for (batch b, q-chunk qc); returns (finish, ot_sb)
    where finish() emits the softmax normalization (deferred into the NEXT
    chunk's pipeline via its `pending` argument, hiding the serial
    recip/broadcast chain under the next chunk's first iterations).

    Software-pipelined: PV(i-1) is emitted after scores(i), so the
    exp+mul latency of iteration i-1 hides under iteration i's score
    matmuls and the PE queue never stalls a full exp round-trip.

    `fillers` are closures emitting independent PE work (projection /
    Wo-projection chains); one is woven in per iteration (between the two
    heads' score matmuls) to fill PE slack while ScalarE paces the loop
    (keeps HAM at 8/8).
    """
    io = g.io
    fillers = list(fillers)
    base = 1
    sched = {}
    for idx, f in enumerate(fillers):
        sched.setdefault(base + idx * (NKS - base) // max(len(fillers), 1),
                         []).append(f)
    qs = b * S + qc * QC
    ps_oA = g.psum_pool.tile([P, QC], f32, tag="ps_o", bufs=1, name=f"psoA{b}_{qc}")
    ps_oB = g.psum_pool.tile([P, QC], f32, tag="ps_d", bufs=1, name=f"psoB{b}_{qc}")
    p_tiles = [None] * NKS
    if (b, qc, 0) not in g.mgrp:
        _expbm_dma(nc, g, b, qc, 0)

    def emit_front(ks, filler=None):
        """scores -> exp -> mask-mul for iteration ks. The filler (an
        independent PE chain) is emitted between head A's and head B's
        score matmuls: it covers exp(A)'s read latency, which head B's
        psum-slot reuse waits on (ps_s ring, bufs=2)."""
        kslice = b * S + ks * P
        # expbm prefetched one k-group (2 iterations) ahead
        kg, ki = ks // 2, ks % 2
        if ki == 0 and kg + 1 < NKS // 2:
            _expbm_dma(nc, g, b, qc, kg + 1)
        g.mAg, g.mBg = g.mgrp[(b, qc, kg)]
        # scores^T for both heads (row-packed, K=64)
        ps_sA = g.psum_pool.tile([P, QC], f32, tag="ps_s", bufs=2,
                                 name=f"pssA{b}_{qc}_{ks}")
        for ch in range(QC // 512):
            nc.tensor.matmul(
                ps_sA[:, ts(ch, 512)],
                g.kT_sb[0:DK, kslice:kslice + P],
                g.qT_sb[0:DK, qs + ch * 512:qs + (ch + 1) * 512],
                start=True, stop=True,
            )
        eA = g.work_pool.tile([P, QC], bf, tag="e", bufs=6, name=f"eA{b}_{qc}_{ks}")
        nc.scalar.activation(eA[:], ps_sA[:], mybir.ActivationFunctionType.Exp)
        pA = g.work_pool.tile([P, QC], bf, tag="p", bufs=6, name=f"pA{b}_{qc}_{ks}")
        nc.vector.tensor_mul(pA[:], eA[:], g.mAg[:, ki, :])
        if filler is not None:
            filler()
        ps_sB = g.psum_pool.tile([P, QC], f32, tag="ps_s", bufs=2,
                                 name=f"pssB{b}_{qc}_{ks}")
        for ch in range(QC // 512):
            nc.tensor.matmul(
                ps_sB[:, ts(ch, 512)],
                g.kT_sb[DK:P, kslice:kslice + P],
                g.qT_sb[DK:P, qs + ch * 512:qs + (ch + 1) * 512],
                start=True, stop=True,
            )
        eB = g.work_pool.tile([P, QC], bf, tag="e", bufs=6, name=f"eB{b}_{qc}_{ks}")
        nc.scalar.activation(eB[:], ps_sB[:], mybir.ActivationFunctionType.Exp)
        pB = g.work_pool.tile([P, QC], bf, tag="p", bufs=6, name=f"pB{b}_{qc}_{ks}")
        nc.vector.tensor_mul(pB[:], eB[:], g.mBg[:, ki, :])
        p_tiles[ks] = (pA, pB)

    def emit_pv(ks):
        vt = (b * S + ks * P) // P
        pA, pB = p_tiles[ks]
        first = ks == 0
        last = ks == NKS - 1
        for ch in range(QC // 512):
            sl = ts(ch, 512)
            # PV with ones-augmented V: lhsT = [v_h | 1] (M=65); rows 0:64 =
            # OT_h, row 64 = softmax denominator — no separate d matmuls.
            nc.tensor.matmul(
                ps_oA[0:DK + 1, sl], g.v_sb[:, vt, 0:DK + 1], pA[:, sl],
                start=first, stop=last,
            )
            nc.tensor.matmul(
                ps_oB[0:DK + 1, sl], g.v_sb[:, vt, VB:VB + DK + 1], pB[:, sl],
                start=first, stop=last,
            )
        p_tiles[ks] = None

    emit_front(0)
    for ks in range(1, NKS):
        if ks == 1 and pending is not None:
            # previous chunk's normalization: its recip/cast chain (DVE)
            # starts under this chunk's first score matmuls instead of
            # stalling the PE at the chunk boundary. Emitted before this
            # iteration's filler — fillers may read the pending ot.
            pending()
        fs = sched.get(ks, ())
        emit_front(ks, filler=fs[0] if fs else None)
        emit_pv(ks - 1)
        for f in fs[1:]:
            f()
        if ks == NKS - 2 and prefetch is not None:
            prefetch()
    emit_pv(NKS - 1)
    # normalize: OTn_h = OT_h * (1/d_h). Reciprocal runs on the full
    # base-0 [65, QC] tile (row 64 = 1/d); the 1/d row is broadcast across
    # 64 partitions with a K=1 matmul whose operands both sit at base 64.
    ot_sb = g.work_pool.tile([P, QC], bf, tag="ot", bufs=2, name=f"ot{b}_{qc}")
    otB_t = g.work_pool.tile([DK, QC], bf, tag="otB", bufs=2, name=f"otB{b}_{qc}")

    def finish(sl=slice(0, QC)):
        n512 = (sl.stop - sl.start) // 512
        for hi, ps_oX in enumerate((ps_oA, ps_oB)):
            r65 = g.work_pool.tile([65, QC], f32, tag="r65", bufs=2,
                                   name=f"r65_{b}_{qc}_{hi}_{sl.start}")
            nc.vector.reciprocal_approx_fast(r65[:, sl], ps_oX[0:65, sl])
            r65b = g.work_pool.tile([65, QC], bf, tag="r65b", bufs=2,
                                    name=f"r65b_{b}_{qc}_{hi}_{sl.start}")
            nc.vector.tensor_copy(r65b[:, sl], r65[:, sl])
            ps_r = g.psum_pool.tile([DK, QC], f32, tag="ps_s", bufs=2,
                                    name=f"psr{b}_{qc}_{hi}_{sl.start}")
            for ch in range(n512):
                cs = slice(sl.start + ch * 512, sl.start + (ch + 1) * 512)
                nc.tensor.matmul(
                    ps_r[:, cs],
                    g.ones65_sb[DK:DK + 1, :],
                    r65b[DK:DK + 1, cs],
                    start=True, stop=True,
                )
            rb_sb = g.work_pool.tile([DK, QC], f32, tag="rbs", bufs=2,
                                     name=f"rbs{b}_{qc}_{hi}_{sl.start}")
            nc.vector.tensor_copy(rb_sb[:, sl], ps_r[:, sl])
            dst = ot_sb[0:DK, sl] if hi == 0 else otB_t[0:DK, sl]
            nc.vector.tensor_mul(dst, ps_oX[0:DK, sl], rb_sb[:, sl])
        # partition-shift hop on the scalar hwdge queue (SWDGE descgen on
        # gpsimd adds ~5 us of latency at the tail)
        nc.scalar.dma_start(ot_sb[DK:P, sl], otB_t[0:DK, sl])

    if tail_factory is not None:
        yp = tail_factory(ot_sb)
        finish(slice(0, 512))
        for f in yp[:4]:
            f()
        finish(slice(512, QC))
        for f in yp[4:]:
            f()
    return finish, ot_sb


def _yproj_chains(nc, g, b, qc, ot_sb):
    """Per-qsub closures for the output projection y[q, :] = OTn[:, q].T @
    WoT — used as PE filler inside later attention chunks."""
    io = g.io
    qs = b * S + qc * QC

    def make(qsub):
        def chain():
            ps_y = g.psum_pool.tile([P, QC], f32, tag="ps_s", bufs=2,
                                    name=f"psy{b}_{qc}_{qsub}")
            for ch in range(D // 512):
                nc.tensor.matmul(
                    ps_y[:, ts(ch, 512)],
                    ot_sb[:, ts(qsub, P)],
                    g.wo_sb[:, ts(ch, 512)],
                    start=True, stop=True,
                )
            y_sb = g.work_pool.tile([P, D], bf, tag="ysb", bufs=3,
                                    name=f"ysb{b}_{qc}_{qsub}")
            if qsub % 2 == 0:
                nc.scalar.copy(y_sb[:], ps_y[:])
            else:
                nc.vector.tensor_copy(y_sb[:], ps_y[:])
            nc.sync.dma_start(io["y"][qs + qsub * P:qs + (qsub + 1) * P, :],
                              y_sb[:])
        return chain

    return [make(qsub) for qsub in range(QC // P)]


def _build_body(nc, tc, io):
    from contextlib import ExitStack
    ctx = ExitStack()
    g = _Ctx()
    g.io = io
    g.xw = {}
    g.mgrp = {}
    g.const_pool = ctx.enter_context(tc.tile_pool(name="const", bufs=1))
    g.stream_pool = ctx.enter_context(tc.tile_pool(name="stream", bufs=3))
    g.work_pool = ctx.enter_context(tc.tile_pool(name="work", bufs=2))
    g.psum_pool = ctx.enter_context(tc.tile_pool(name="psum", bufs=2, space="PSUM"))

    # ---- persistent SBUF tensors ----
    g.wq_sb = g.const_pool.tile([P, KO, DKC], bf, tag="wq", name="wq_sb")
    g.wk_sb = g.const_pool.tile([P, KO, DKC], bf, tag="wk", name="wk_sb")
    g.wv_sb = g.const_pool.tile([P, KO, DKC], bf, tag="wv", name="wv_sb")
    g.wo_sb = g.const_pool.tile([P, D], bf, tag="wo", name="wo_sb")
    g.ident_sb = g.const_pool.tile([P, P], bf, tag="ident", name="ident_sb")
    make_identity(nc, g.ident_sb[:])
    g.ones65_sb = g.const_pool.tile([65, DK], bf, tag="ones65", name="ones65_sb")
    nc.vector.memset(g.ones65_sb[:], 1.0)

    g.qT_sb = g.const_pool.tile([P, T], bf, tag="qT", name="qT_sb")
    g.kT_sb = g.const_pool.tile([P, T], bf, tag="kT", name="kT_sb")
    # v layout per 128-token tile: [vA(64) | 1 | pad | vB(64) | 1 | pad] for
    # ones-aug PV; vB at a 32B-aligned offset so DMA-transpose writes land
    # directly. Full-tile memset; transposes overwrite all but ones/pad.
    g.v_sb = g.const_pool.tile([P, T // P, VW], bf, tag="v", name="v_sb")
    nc.vector.memset(g.v_sb[:], 1.0)

    # Startup DMA order = exact consumption order of the c0 chains (k, v,
    # q): each weight immediately before the x stream its chain consumes,
    # so the first k matmul waits on only ~768 KiB of queue, not 2 MiB.
    for wt, wsrc, tag, xsrc in (
            (g.wk_sb, "wkT", "xk", "xkT"),
            (g.wv_sb, "wvT", "xv", "xvT"),
            (g.wq_sb, "wqT", "xq", "xqT")):
        nc.sync.dma_start(wt[:], io[wsrc])
        t = g.stream_pool.tile([P, KO, 2 * TC], bf, tag=tag, bufs=2,
                               name=f"{tag}w0")
        for ko in range(KO):
            nc.sync.dma_start(t[:, ko, :], io[xsrc][0, :, ko, :])
        g.xw[(tag, 0)] = t
    nc.sync.dma_start(g.wo_sb[:], io["woT"])

    # Emission plan: only c0/c1's k+q chains run up-front (att(0,0) needs
    # qT[0:1024] and kT progressively); every other chain — including the
    # c0/c1 v-chains — is woven into an attention chunk as PE filler so
    # TensorE never idles long enough for HAM to re-throttle. Each chunk's
    # serial normalization is deferred into the next chunk's pipeline.
    # Dependency alignment: att(0,0) consumes c2/c3 from ks>=8 (woven by
    # iter ~7) and v(c0)/v(c1) from PV(0)/PV(4) (first two fillers);
    # att(1,0) consumes c6/c7 from ks>=8 (woven by iter ~5).
    _expbm_dma(nc, g, 0, 0, 0)
    _load_xpair(nc, g, 1)
    ch = {c: _proj_chains(nc, g, c) for c in range(8)}
    kc, vc, qc_ = 0, 1, 2
    for f in ch[0]:
        f()
    for f in ch[1]:
        f()

    _load_xpair(nc, g, 2)
    f00, ot00 = _attention_chunk(
        nc, g, 0, 0,
        fillers=ch[2] + ch[3] + ch[4],
        prefetch=lambda: _expbm_dma(nc, g, 0, 1, 0))
    _load_xpair(nc, g, 3)
    f01, ot01 = _attention_chunk(
        nc, g, 0, 1,
        fillers=ch[5] + _yproj_chains(nc, g, 0, 0, ot00),
        pending=f00,
        prefetch=lambda: _expbm_dma(nc, g, 1, 0, 0))
    f10, ot10 = _attention_chunk(
        nc, g, 1, 0,
        fillers=[ch[6][kc], ch[6][qc_], ch[6][vc],
                 ch[7][kc], ch[7][qc_], ch[7][vc]]
        + _yproj_chains(nc, g, 0, 1, ot01),
        pending=f01,
        prefetch=lambda: _expbm_dma(nc, g, 1, 1, 0))
    _attention_chunk(nc, g, 1, 1,
                     fillers=_yproj_chains(nc, g, 1, 0, ot10),
                     pending=f10,
                     tail_factory=lambda ot: _yproj_chains(nc, g, 1, 1, ot))

    ctx.close()


def build_nc():
    nc = bacc.Bacc("TRN2", target_bir_lowering=False, debug=False,
                   num_devices=N_CORES)
    NCW = T // (2 * TC)        # 4 double-width x chunks
    io = {
        # x pre-tiled host-side: [chunk, ki, ko, 2*TC] — contiguous per DMA
        "xqT": nc.dram_tensor("xqT", [NCW, P, KO, 2 * TC], bf,
                              kind="ExternalInput").ap(),
        "xkT": nc.dram_tensor("xkT", [NCW, P, KO, 2 * TC], bf,
                              kind="ExternalInput").ap(),
        "xvT": nc.dram_tensor("xvT", [NCW, P, KO, 2 * TC], bf,
                              kind="ExternalInput").ap(),
        "wqT": nc.dram_tensor("wqT", [P, KO, DKC], bf, kind="ExternalInput").ap(),
        "wkT": nc.dram_tensor("wkT", [P, KO, DKC], bf, kind="ExternalInput").ap(),
        "wvT": nc.dram_tensor("wvT", [P, KO, DKC], bf, kind="ExternalInput").ap(),
        "woT": nc.dram_tensor("woT", [DKC, D], bf, kind="ExternalInput").ap(),
        # expbm pre-tiled: [b, h, qc, kg, ki, 2, QC] — contiguous per DMA
        "expbm": nc.dram_tensor("expbm", [B, HC, NQC, NKS // 2, P, 2, QC], bf,
                                kind="ExternalInput").ap(),
        "y": nc.dram_tensor("y", [T, D], bf, kind="ExternalOutput").ap(),
    }
    with tile.TileContext(nc) as tc:
        _build_body(nc, tc, io)
    nc.compile()
    return nc


_NC_CACHE = None


def _get_nc():
    global _NC_CACHE
    if _NC_CACHE is None:
        _NC_CACHE = build_nc()
    return _NC_CACHE


def make_in_maps(query, key, value, mask, rel_pos_bias,
                 Wq, bq, Wk, bk, Wv, bv, Wo, bo):
    """Host-side sharding/preprocessing -> per-core input dicts."""
    NCW = T // (2 * TC)

    def tile_x(x):
        # [T, D] -> [NCW, P, KO, 2*TC]: xt[cw, ki, ko, t] = x[cw*1024+t, ko*128+ki]
        return np.ascontiguousarray(
            x.reshape(NCW, 2 * TC, KO, P).transpose(0, 3, 2, 1).astype(BF16))

    xqT = tile_x(query.reshape(T, D))
    xkT = tile_x(key.reshape(T, D))
    xvT = tile_x(value.reshape(T, D))

    def tile_w(wT):
        # [D, DKC] -> [P, KO, DKC]
        return np.ascontiguousarray(wT.reshape(KO, P, DKC).transpose(1, 0, 2))

    def tile_expbm(e):
        # [B, HC, S(k), S(q)] -> [B, HC, NQC, NKS//2, P, 2, QC]
        return np.ascontiguousarray(
            e.reshape(B, HC, NKS // 2, 2, P, NQC, QC)
             .transpose(0, 1, 5, 2, 4, 3, 6))

    scale = 1.0 / np.sqrt(np.float32(DK))
    maskinv = (~mask[:, 0]).astype(np.float32)          # [B, Sq, Sk]

    # bq/bk handling: scores_full = (q+bq)(k+bk)^T * scale.
    # The (q'+bq)·bk term varies only along q => softmax-invariant, dropped.
    # The bq·(k'+bk) term varies along k; fold exp(delta_k) into expbm when
    # bq is nonzero (needs host k-projection).
    need_delta = bool(np.any(bq))
    if need_delta:
        k_proj = key.reshape(T, D).astype(np.float32) @ Wk.T.astype(np.float32) + bk

    in_maps = []
    for c in range(N_CORES):
        hs = slice(c * DKC, (c + 1) * DKC)
        wqT = tile_w((Wq[hs, :] * scale).T.astype(BF16))
        wkT = tile_w(Wk[hs, :].T.astype(BF16))
        wvT = tile_w(Wv[hs, :].T.astype(BF16))
        woT = np.ascontiguousarray(Wo[:, hs].T.astype(BF16))
        expbm = np.empty((B, HC, S, S), dtype=BF16)
        for hi in range(HC):
            h = c * HC + hi
            ebT = np.exp(rel_pos_bias[0, h].astype(np.float32)).T  # [k, q]
            if need_delta:
                delta = scale * (
                    k_proj[:, h * DK:(h + 1) * DK] @ bq[h * DK:(h + 1) * DK]
                    + np.dot(bq[h * DK:(h + 1) * DK], bk[h * DK:(h + 1) * DK])
                )  # [T] along k
                for b_ in range(B):
                    ebb = ebT * np.exp(delta[b_ * S:(b_ + 1) * S])[:, None]
                    expbm[b_, hi] = (ebb * maskinv[b_].T).astype(BF16)
            else:
                for b_ in range(B):
                    expbm[b_, hi] = (ebT * maskinv[b_].T).astype(BF16)
        in_maps.append({
            "xqT": xqT, "xkT": xkT, "xvT": xvT,
            "wqT": wqT, "wkT": wkT, "wvT": wvT, "woT": woT,
            "expbm": tile_expbm(expbm),
        })
    return in_maps


def assemble_output(results, value_bias, Wo, bo):
    out = np.zeros((T, D), np.float32)
    for r in results:
        out += r["y"].astype(np.float32)
    # exact bv contribution: softmax rows sum to 1 => attn_out += bv,
    # so y += bv @ Wo^T; plus bo.
    out += value_bias.astype(np.float32) @ Wo.T.astype(np.float32)
    out += bo.astype(np.float32)[None, :]
    return out.reshape(B, S, D)


def kernel(query, key, value, mask, rel_pos_bias,
           Wq, bq, Wk, bk, Wv, bv, Wo, bo, _run_kwargs=None):
    query = np.asarray(query); key = np.asarray(key); value = np.asarray(value)
    mask = np.asarray(mask); rel_pos_bias = np.asarray(rel_pos_bias)
    Wq = np.asarray(Wq); Wk = np.asarray(Wk); Wv = np.asarray(Wv)
    Wo = np.asarray(Wo)
    bq = np.asarray(bq); bk = np.asarray(bk); bv = np.asarray(bv)
    bo = np.asarray(bo)

    nc = _get_nc()
    in_maps = make_in_maps(query, key, value, mask, rel_pos_bias,
                           Wq, bq, Wk, bk, Wv, bv, Wo, bo)
    kw = _run_kwargs or {}
    res = run_bass_kernel_spmd(nc, in_maps, core_ids=list(range(N_CORES)), **kw)
    out = assemble_output(res.results, bv, Wo, bo)
    if _run_kwargs is not None:
        kernel._last_results = res
    return out



# revision 41
# speedup vs baseline: 1.1933x; 1.1933x over previous
"""Trainium2 Bass kernel for nn_MultiHeadedAttention (B=2,S=2048,D=1024,H=16).

Sharding: tensor-parallel over heads — 2 heads per core x 8 cores.
Each core computes its 2 heads' attention and a partial output projection
(y_partial [B*S, D], bf16); the host sums the 8 partials and adds bo.

Device pipeline per core (all matmul operands bf16, fp32 PSUM accumulate):
  qT/kT = W @ xT (feature-major), v = x @ WvT (feature-major + PE transpose)
  S^T[k,q] = K @ Q^T (row-packed 2 heads, contraction dk=64)
  p = exp(S^T) * expbm   (expbm = exp(bias)*(1-mask), host-precomputed, bf16)
  OT = V^T @ p (ones-augmented V: row 64 = softmax denominator)
  OTn = OT * recip(d) -> y_partial = OTn^T @ WoT

Scheduling (the performance-critical part — TensorE must stream gap-free or
the HAM clock-gate drops it from 2.0 GHz to 1.2 GHz):
  - attention inner loop is software-pipelined: PV(i-1) is emitted after
    scores(i) so the exp+mul latency hides under the next score matmuls;
  - every projection / Wo-projection chain is woven INTO the attention
    loops as PE filler, placed between the two heads' score matmuls where
    it also covers exp(A)'s PSUM-slot latency (ps_s ring, bufs=2);
  - DRAM layouts are host-pre-tiled so every DMA is large + contiguous
    (x: 2 MiB per transfer, expbm: 512 KiB per transfer).
"""

import numpy as np
import ml_dtypes

import concourse.bass as bass
import concourse.tile as tile
from concourse import bacc, mybir
from concourse.bass import ts
from concourse.bass_utils import run_bass_kernel_spmd
from concourse.masks import make_identity

BF16 = ml_dtypes.bfloat16

B, S, D, H = 2, 2048, 1024, 16
N_CORES = 8
HC = H // N_CORES          # heads per core = 2
DK = D // H                # 64
DKC = HC * DK              # head dims per core = 128
P = 128
T = B * S                  # 4096 tokens
KO = D // P                # 8 feature k-subtiles
TC = 512                   # token chunk for projections
QC = 1024                  # q chunk for attention phase
NKS = S // P               # 16 k-subtiles per batch
NQC = S // QC              # 2 q-chunks per batch

bf = mybir.dt.bfloat16
f32 = mybir.dt.float32

VB = DK + 1                # vB column offset in v_sb
VW = DKC + 2               # v_sb row width


class _Ctx:
    pass


def _load_xpair(nc, g, p, fine=False):
    """Prefetch the double-width (1024-token) x tiles for chunk pair p.
    Host pre-tiles the layout so each DMA is one fully-contiguous 2 MiB
    block (16 KiB per partition). fine=True splits into per-ko 512 KiB
    sub-DMAs in consumption order so the first projection matmul can
    start ~1 us in instead of waiting for the full 6 MiB."""
    io = g.io
    for tag, src in (("xk", "xkT"), ("xv", "xvT"), ("xq", "xqT")):
        t = g.stream_pool.tile([P, KO, 2 * TC], bf, tag=tag, bufs=2,
                               name=f"{tag}w{p}")
        if fine:
            for ko in range(KO):
                nc.sync.dma_start(t[:, ko, :], io[src][p, :, ko, :])
        else:
            nc.sync.dma_start(t[:], io[src][p])
        g.xw[(tag, p)] = t


def _expbm_dma(nc, g, b, qc, kg):
    """Issue the expbm DMA for k-group kg (2 k-subtiles x QC per head) on
    the sync queue, one fully-contiguous 512 KiB transfer per head. Kept
    OFF the scalar queue: the exp ACTIVATEs pace the whole pipeline and
    must not sit behind DMA triggers."""
    io = g.io
    mA = g.work_pool.tile([P, 2, QC], bf, tag="mA", bufs=2,
                          name=f"mAg{b}_{qc}_{kg}")
    nc.sync.dma_start(mA[:], io["expbm"][b, 0, qc, kg])
    mB = g.work_pool.tile([P, 2, QC], bf, tag="mB", bufs=2,
                          name=f"mBg{b}_{qc}_{kg}")
    nc.sync.dma_start(mB[:], io["expbm"][b, 1, qc, kg])
    g.mgrp[(b, qc, kg)] = (mA, mB)


def _proj_chains(nc, g, c):
    """Build [k, v, q] emission closures for projection chunk c (512 tokens).
    Each closure is ~1.7-2.5us of dense PE work, used as PE filler inside
    attention chunks so the TensorE stream never gaps (keeps HAM at 8/8)."""
    h = c % 2
    p = c // 2

    def k_chain():
        ps_k = g.psum_pool.tile([P, QC], f32, tag="ps_s", bufs=2, name=f"psk{c}")
        for ko in range(KO):
            nc.tensor.matmul(
                ps_k[:, :TC], g.wk_sb[:, ko, :],
                g.xw[("xk", p)][:, ko, ts(h, TC)],
                start=(ko == 0), stop=(ko == KO - 1),
            )
        nc.vector.tensor_copy(g.kT_sb[:, ts(c, TC)], ps_k[:, :TC])

    def v_chain():
        # v computed feature-major (dense N=512 matmuls), then moved into
        # the token-major layout PV needs via the DMA transpose engine —
        # zero TensorE / VectorE cost beyond the PSUM->SBUF cast.
        ps_vT = g.psum_pool.tile([P, QC], f32, tag="ps_s", bufs=2,
                                 name=f"psvT{c}")
        for ko in range(KO):
            nc.tensor.matmul(
                ps_vT[:, :TC], g.wv_sb[:, ko, :],
                g.xw[("xv", p)][:, ko, ts(h, TC)],
                start=(ko == 0), stop=(ko == KO - 1),
            )
        vt_tmp = g.work_pool.tile([P, TC], bf, tag="vtt", bufs=2, name=f"vtt{c}")
        nc.vector.tensor_copy(vt_tmp[:], ps_vT[:, :TC])
        for tt in range(TC // P):
            ps_t = g.psum_pool.tile([P, P], bf, tag="ps_s", bufs=2,
                                    name=f"pst{c}_{tt}")
            nc.tensor.transpose(ps_t[:], vt_tmp[:, ts(tt, P)], g.ident_sb[:])
            vt_i = c * (TC // P) + tt
            nc.vector.tensor_copy(g.v_sb[:, vt_i, 0:DK], ps_t[:, 0:DK])
            nc.vector.tensor_copy(g.v_sb[:, vt_i, VB:VB + DK], ps_t[:, DK:DKC])

    def q_chain():
        ps_q = g.psum_pool.tile([P, QC], f32, tag="ps_s", bufs=2, name=f"psq{c}")
        for ko in range(KO):
            nc.tensor.matmul(
                ps_q[:, :TC], g.wq_sb[:, ko, :],
                g.xw[("xq", p)][:, ko, ts(h, TC)],
                start=(ko == 0), stop=(ko == KO - 1),
            )
        nc.vector.tensor_copy(g.qT_sb[:, ts(c, TC)], ps_q[:, :TC])

    return [k_chain, v_chain, q_chain]


def _attention_chunk(nc, g, b, qc, fillers=(), pending=None, prefetch=None,
                     tail_factory=None):
    """Emit attention for (batch b, q-chunk qc); returns (finish, ot_sb)
    where finish() emits the softmax normalization (deferred into the NEXT
    chunk's pipeline via its `pending` argument, hiding the serial
    recip/broadcast chain under the next chunk's first iterations).

    Software-pipelined: PV(i-1) is emitted after scores(i), so the
    exp+mul latency of iteration i-1 hides under iteration i's score
    matmuls and the PE queue never stalls a full exp round-trip.

    `fillers` are closures emitting independent PE work (projection /
    Wo-projection chains); one is woven in per iteration (between the two
    heads' score matmuls) to fill PE slack while ScalarE paces the loop
    (keeps HAM at 8/8).
    """
    io = g.io
    fillers = list(fillers)
    # with a pending normalization, fillers start at slot 2: pending is
    # emitted in slot 1 (after scores(1)) and a slot-1 filler could read
    # the ot tile pending writes (the final chunk's yproj fillers do).
    base = 2 if pending is not None else 1
    sched = {}
    for idx, f in enumerate(fillers):
        sched.setdefault(base + idx * (NKS - base) // max(len(fillers), 1),
                         []).append(f)
    qs = b * S + qc * QC
    ps_oA = g.psum_pool.tile([P, QC], f32, tag="ps_o", bufs=1, name=f"psoA{b}_{qc}")
    ps_oB = g.psum_pool.tile([P, QC], f32, tag="ps_d", bufs=1, name=f"psoB{b}_{qc}")
    p_tiles = [None] * NKS
    if (b, qc, 0) not in g.mgrp:
        _expbm_dma(nc, g, b, qc, 0)

    def emit_front(ks, filler=None):
        """scores -> exp -> mask-mul for iteration ks. The filler (an
        independent PE chain) is emitted between head A's and head B's
        score matmuls: it covers exp(A)'s read latency, which head B's
        psum-slot reuse waits on (ps_s ring, bufs=2)."""
        kslice = b * S + ks * P
        # expbm prefetched one k-group (2 iterations) ahead
        kg, ki = ks // 2, ks % 2
        if ki == 0 and kg + 1 < NKS // 2:
            _expbm_dma(nc, g, b, qc, kg + 1)
        g.mAg, g.mBg = g.mgrp[(b, qc, kg)]
        # scores^T for both heads (row-packed, K=64)
        ps_sA = g.psum_pool.tile([P, QC], f32, tag="ps_s", bufs=2,
                                 name=f"pssA{b}_{qc}_{ks}")
        for ch in range(QC // 512):
            nc.tensor.matmul(
                ps_sA[:, ts(ch, 512)],
                g.kT_sb[0:DK, kslice:kslice + P],
                g.qT_sb[0:DK, qs + ch * 512:qs + (ch + 1) * 512],
                start=True, stop=True,
            )
        eA = g.work_pool.tile([P, QC], bf, tag="e", bufs=6, name=f"eA{b}_{qc}_{ks}")
        nc.scalar.activation(eA[:], ps_sA[:], mybir.ActivationFunctionType.Exp)
        pA = g.work_pool.tile([P, QC], bf, tag="p", bufs=6, name=f"pA{b}_{qc}_{ks}")
        nc.vector.tensor_mul(pA[:], eA[:], g.mAg[:, ki, :])
        if filler is not None:
            filler()
        ps_sB = g.psum_pool.tile([P, QC], f32, tag="ps_s", bufs=2,
                                 name=f"pssB{b}_{qc}_{ks}")
        for ch in range(QC // 512):
            nc.tensor.matmul(
                ps_sB[:, ts(ch, 512)],
                g.kT_sb[DK:P, kslice:kslice + P],
                g.qT_sb[DK:P, qs + ch * 512:qs + (ch + 1) * 512],
                start=True, stop=True,
            )
        eB = g.work_pool.tile([P, QC], bf, tag="e", bufs=6, name=f"eB{b}_{qc}_{ks}")
        nc.scalar.activation(eB[:], ps_sB[:], mybir.ActivationFunctionType.Exp)
        pB = g.work_pool.tile([P, QC], bf, tag="p", bufs=6, name=f"pB{b}_{qc}_{ks}")
        nc.vector.tensor_mul(pB[:], eB[:], g.mBg[:, ki, :])
        p_tiles[ks] = (pA, pB)

    def emit_pv(ks):
        vt = (b * S + ks * P) // P
        pA, pB = p_tiles[ks]
        first = ks == 0
        last = ks == NKS - 1
        for ch in range(QC // 512):
            sl = ts(ch, 512)
            # PV with ones-augmented V: lhsT = [v_h | 1] (M=65); rows 0:64 =
            # OT_h, row 64 = softmax denominator — no separate d matmuls.
            nc.tensor.matmul(
                ps_oA[0:DK + 1, sl], g.v_sb[:, vt, 0:DK + 1], pA[:, sl],
                start=first, stop=last,
            )
            nc.tensor.matmul(
                ps_oB[0:DK + 1, sl], g.v_sb[:, vt, VB:VB + DK + 1], pB[:, sl],
                start=first, stop=last,
            )
        p_tiles[ks] = None

    emit_front(0)
    for ks in range(1, NKS):
        fs = sched.get(ks, ())
        emit_front(ks, filler=fs[0] if fs else None)
        if ks == 1 and pending is not None:
            # previous chunk's normalization: emitted after scores(0..1)
            # so its PE bcast matmuls queue behind ~2 iterations of score
            # work — full cover for the serial recip/cast chain (DVE).
            pending()
        emit_pv(ks - 1)
        for f in fs[1:]:
            f()
        if ks == NKS - 2 and prefetch is not None:
            prefetch()
    emit_pv(NKS - 1)
    # normalize: OTn_h = OT_h * (1/d_h). Reciprocal runs on the full
    # base-0 [65, QC] tile (row 64 = 1/d); the 1/d row is broadcast across
    # 64 partitions with a K=1 matmul whose operands both sit at base 64.
    ot_sb = g.work_pool.tile([P, QC], bf, tag="ot", bufs=2, name=f"ot{b}_{qc}")
    otB_t = g.work_pool.tile([DK, QC], bf, tag="otB", bufs=2, name=f"otB{b}_{qc}")

    def finish(sl=slice(0, QC)):
        n512 = (sl.stop - sl.start) // 512
        for hi, ps_oX in enumerate((ps_oA, ps_oB)):
            r65 = g.work_pool.tile([65, QC], f32, tag="r65", bufs=2,
                                   name=f"r65_{b}_{qc}_{hi}_{sl.start}")
            nc.vector.reciprocal_approx_fast(r65[:, sl], ps_oX[0:65, sl])
            r65b = g.work_pool.tile([65, QC], bf, tag="r65b", bufs=2,
                                    name=f"r65b_{b}_{qc}_{hi}_{sl.start}")
            nc.vector.tensor_copy(r65b[:, sl], r65[:, sl])
            ps_r = g.psum_pool.tile([DK, QC], f32, tag="ps_s", bufs=2,
                                    name=f"psr{b}_{qc}_{hi}_{sl.start}")
            for ch in range(n512):
                cs = slice(sl.start + ch * 512, sl.start + (ch + 1) * 512)
                nc.tensor.matmul(
                    ps_r[:, cs],
                    g.ones65_sb[DK:DK + 1, :],
                    r65b[DK:DK + 1, cs],
                    start=True, stop=True,
                )
            rb_sb = g.work_pool.tile([DK, QC], f32, tag="rbs", bufs=2,
                                     name=f"rbs{b}_{qc}_{hi}_{sl.start}")
            nc.vector.tensor_copy(rb_sb[:, sl], ps_r[:, sl])
            dst = ot_sb[0:DK, sl] if hi == 0 else otB_t[0:DK, sl]
            nc.vector.tensor_mul(dst, ps_oX[0:DK, sl], rb_sb[:, sl])
        # partition-shift hop on the scalar hwdge queue (SWDGE descgen on
        # gpsimd adds ~5 us of latency at the tail)
        nc.scalar.dma_start(ot_sb[DK:P, sl], otB_t[0:DK, sl])

    if tail_factory is not None:
        yp = tail_factory(ot_sb)
        finish(slice(0, 512))
        for f in yp[:4]:
            f()
        finish(slice(512, QC))
        for f in yp[4:]:
            f()
    return finish, ot_sb


def _yproj_chains(nc, g, b, qc, ot_sb):
    """Per-qsub closures for the output projection y[q, :] = OTn[:, q].T @
    WoT — used as PE filler inside later attention chunks."""
    io = g.io
    qs = b * S + qc * QC

    def make(qsub):
        def chain():
            ps_y = g.psum_pool.tile([P, QC], f32, tag="ps_s", bufs=2,
                                    name=f"psy{b}_{qc}_{qsub}")
            for ch in range(D // 512):
                nc.tensor.matmul(
                    ps_y[:, ts(ch, 512)],
                    ot_sb[:, ts(qsub, P)],
                    g.wo_sb[:, ts(ch, 512)],
                    start=True, stop=True,
                )
            y_sb = g.work_pool.tile([P, D], bf, tag="ysb", bufs=3,
                                    name=f"ysb{b}_{qc}_{qsub}")
            if qsub % 2 == 0:
                nc.scalar.copy(y_sb[:], ps_y[:])
            else:
                nc.vector.tensor_copy(y_sb[:], ps_y[:])
            nc.sync.dma_start(io["y"][qs + qsub * P:qs + (qsub + 1) * P, :],
                              y_sb[:])
        return chain

    return [make(qsub) for qsub in range(QC // P)]


def _build_body(nc, tc, io):
    from contextlib import ExitStack
    ctx = ExitStack()
    g = _Ctx()
    g.io = io
    g.xw = {}
    g.mgrp = {}
    g.const_pool = ctx.enter_context(tc.tile_pool(name="const", bufs=1))
    g.stream_pool = ctx.enter_context(tc.tile_pool(name="stream", bufs=3))
    g.work_pool = ctx.enter_context(tc.tile_pool(name="work", bufs=2))
    g.psum_pool = ctx.enter_context(tc.tile_pool(name="psum", bufs=2, space="PSUM"))

    # ---- persistent SBUF tensors ----
    g.wq_sb = g.const_pool.tile([P, KO, DKC], bf, tag="wq", name="wq_sb")
    nc.sync.dma_start(g.wq_sb[:], io["wqT"])
    g.wk_sb = g.const_pool.tile([P, KO, DKC], bf, tag="wk", name="wk_sb")
    nc.sync.dma_start(g.wk_sb[:], io["wkT"])
    g.wv_sb = g.const_pool.tile([P, KO, DKC], bf, tag="wv", name="wv_sb")
    nc.sync.dma_start(g.wv_sb[:], io["wvT"])
    g.wo_sb = g.const_pool.tile([P, D], bf, tag="wo", name="wo_sb")
    nc.sync.dma_start(g.wo_sb[:], io["woT"])
    g.ident_sb = g.const_pool.tile([P, P], bf, tag="ident", name="ident_sb")
    make_identity(nc, g.ident_sb[:])
    g.ones65_sb = g.const_pool.tile([65, DK], bf, tag="ones65", name="ones65_sb")
    nc.vector.memset(g.ones65_sb[:], 1.0)

    g.qT_sb = g.const_pool.tile([P, T], bf, tag="qT", name="qT_sb")
    g.kT_sb = g.const_pool.tile([P, T], bf, tag="kT", name="kT_sb")
    # v layout per 128-token tile: [vA(64) | 1 | pad | vB(64) | 1 | pad] for
    # ones-aug PV; vB at a 32B-aligned offset so DMA-transpose writes land
    # directly. Full-tile memset; transposes overwrite all but ones/pad.
    g.v_sb = g.const_pool.tile([P, T // P, VW], bf, tag="v", name="v_sb")
    nc.vector.memset(g.v_sb[:], 1.0)

    # Emission plan: only c0/c1's k+q chains run up-front (att(0,0) needs
    # qT[0:1024] and kT progressively); every other chain — including the
    # c0/c1 v-chains — is woven into an attention chunk as PE filler so
    # TensorE never idles long enough for HAM to re-throttle. Each chunk's
    # serial normalization is deferred into the next chunk's pipeline.
    # Dependency alignment: att(0,0) consumes c2/c3 from ks>=8 (woven by
    # iter ~7) and v(c0)/v(c1) from PV(0)/PV(4) (first two fillers);
    # att(1,0) consumes c6/c7 from ks>=8 (woven by iter ~5).
    _expbm_dma(nc, g, 0, 0, 0)
    _load_xpair(nc, g, 0, fine=True)
    _load_xpair(nc, g, 1)
    ch = {c: _proj_chains(nc, g, c) for c in range(8)}
    kc, vc, qc_ = 0, 1, 2
    for f in ch[0]:
        f()
    for f in ch[1]:
        f()

    _load_xpair(nc, g, 2)
    f00, ot00 = _attention_chunk(
        nc, g, 0, 0,
        fillers=ch[2] + ch[3] + ch[4],
        prefetch=lambda: _expbm_dma(nc, g, 0, 1, 0))
    _load_xpair(nc, g, 3)
    f01, ot01 = _attention_chunk(
        nc, g, 0, 1,
        fillers=ch[5] + _yproj_chains(nc, g, 0, 0, ot00),
        pending=f00,
        prefetch=lambda: _expbm_dma(nc, g, 1, 0, 0))
    f10, ot10 = _attention_chunk(
        nc, g, 1, 0,
        fillers=[ch[6][kc], ch[6][qc_], ch[6][vc],
                 ch[7][kc], ch[7][qc_], ch[7][vc]]
        + _yproj_chains(nc, g, 0, 1, ot01),
        pending=f01,
        prefetch=lambda: _expbm_dma(nc, g, 1, 1, 0))
    _attention_chunk(nc, g, 1, 1,
                     fillers=_yproj_chains(nc, g, 1, 0, ot10),
                     pending=f10,
                     tail_factory=lambda ot: _yproj_chains(nc, g, 1, 1, ot))

    ctx.close()


def build_nc():
    nc = bacc.Bacc("TRN2", target_bir_lowering=False, debug=False,
                   num_devices=N_CORES)
    NCW = T // (2 * TC)        # 4 double-width x chunks
    io = {
        # x pre-tiled host-side: [chunk, ki, ko, 2*TC] — contiguous per DMA
        "xqT": nc.dram_tensor("xqT", [NCW, P, KO, 2 * TC], bf,
                              kind="ExternalInput").ap(),
        "xkT": nc.dram_tensor("xkT", [NCW, P, KO, 2 * TC], bf,
                              kind="ExternalInput").ap(),
        "xvT": nc.dram_tensor("xvT", [NCW, P, KO, 2 * TC], bf,
                              kind="ExternalInput").ap(),
        "wqT": nc.dram_tensor("wqT", [P, KO, DKC], bf, kind="ExternalInput").ap(),
        "wkT": nc.dram_tensor("wkT", [P, KO, DKC], bf, kind="ExternalInput").ap(),
        "wvT": nc.dram_tensor("wvT", [P, KO, DKC], bf, kind="ExternalInput").ap(),
        "woT": nc.dram_tensor("woT", [DKC, D], bf, kind="ExternalInput").ap(),
        # expbm pre-tiled: [b, h, qc, kg, ki, 2, QC] — contiguous per DMA
        "expbm": nc.dram_tensor("expbm", [B, HC, NQC, NKS // 2, P, 2, QC], bf,
                                kind="ExternalInput").ap(),
        "y": nc.dram_tensor("y", [T, D], bf, kind="ExternalOutput").ap(),
    }
    with tile.TileContext(nc) as tc:
        _build_body(nc, tc, io)
    nc.compile()
    return nc


_NC_CACHE = None


def _get_nc():
    global _NC_CACHE
    if _NC_CACHE is None:
        _NC_CACHE = build_nc()
    return _NC_CACHE


def make_in_maps(query, key, value, mask, rel_pos_bias,
                 Wq, bq, Wk, bk, Wv, bv, Wo, bo):
    """Host-side sharding/preprocessing -> per-core input dicts."""
    NCW = T // (2 * TC)

    def tile_x(x):
        # [T, D] -> [NCW, P, KO, 2*TC]: xt[cw, ki, ko, t] = x[cw*1024+t, ko*128+ki]
        return np.ascontiguousarray(
            x.reshape(NCW, 2 * TC, KO, P).transpose(0, 3, 2, 1).astype(BF16))

    xqT = tile_x(query.reshape(T, D))
    xkT = tile_x(key.reshape(T, D))
    xvT = tile_x(value.reshape(T, D))

    def tile_w(wT):
        # [D, DKC] -> [P, KO, DKC]
        return np.ascontiguousarray(wT.reshape(KO, P, DKC).transpose(1, 0, 2))

    def tile_expbm(e):
        # [B, HC, S(k), S(q)] -> [B, HC, NQC, NKS//2, P, 2, QC]
        return np.ascontiguousarray(
            e.reshape(B, HC, NKS // 2, 2, P, NQC, QC)
             .transpose(0, 1, 5, 2, 4, 3, 6))

    scale = 1.0 / np.sqrt(np.float32(DK))
    maskinv = (~mask[:, 0]).astype(np.float32)          # [B, Sq, Sk]

    # bq/bk handling: scores_full = (q+bq)(k+bk)^T * scale.
    # The (q'+bq)·bk term varies only along q => softmax-invariant, dropped.
    # The bq·(k'+bk) term varies along k; fold exp(delta_k) into expbm when
    # bq is nonzero (needs host k-projection).
    need_delta = bool(np.any(bq))
    if need_delta:
        k_proj = key.reshape(T, D).astype(np.float32) @ Wk.T.astype(np.float32) + bk

    in_maps = []
    for c in range(N_CORES):
        hs = slice(c * DKC, (c + 1) * DKC)
        wqT = tile_w((Wq[hs, :] * scale).T.astype(BF16))
        wkT = tile_w(Wk[hs, :].T.astype(BF16))
        wvT = tile_w(Wv[hs, :].T.astype(BF16))
        woT = np.ascontiguousarray(Wo[:, hs].T.astype(BF16))
        expbm = np.empty((B, HC, S, S), dtype=BF16)
        for hi in range(HC):
            h = c * HC + hi
            ebT = np.exp(rel_pos_bias[0, h].astype(np.float32)).T  # [k, q]
            if need_delta:
                delta = scale * (
                    k_proj[:, h * DK:(h + 1) * DK] @ bq[h * DK:(h + 1) * DK]
                    + np.dot(bq[h * DK:(h + 1) * DK], bk[h * DK:(h + 1) * DK])
                )  # [T] along k
                for b_ in range(B):
                    ebb = ebT * np.exp(delta[b_ * S:(b_ + 1) * S])[:, None]
                    expbm[b_, hi] = (ebb * maskinv[b_].T).astype(BF16)
            else:
                for b_ in range(B):
                    expbm[b_, hi] = (ebT * maskinv[b_].T).astype(BF16)
        in_maps.append({
            "xqT": xqT, "xkT": xkT, "xvT": xvT,
            "wqT": wqT, "wkT": wkT, "wvT": wvT, "woT": woT,
            "expbm": tile_expbm(expbm),
        })
    return in_maps


def assemble_output(results, value_bias, Wo, bo):
    out = np.zeros((T, D), np.float32)
    for r in results:
        out += r["y"].astype(np.float32)
    # exact bv contribution: softmax rows sum to 1 => attn_out += bv,
    # so y += bv @ Wo^T; plus bo.
    out += value_bias.astype(np.float32) @ Wo.T.astype(np.float32)
    out += bo.astype(np.float32)[None, :]
    return out.reshape(B, S, D)


def kernel(query, key, value, mask, rel_pos_bias,
           Wq, bq, Wk, bk, Wv, bv, Wo, bo, _run_kwargs=None):
    query = np.asarray(query); key = np.asarray(key); value = np.asarray(value)
    mask = np.asarray(mask); rel_pos_bias = np.asarray(rel_pos_bias)
    Wq = np.asarray(Wq); Wk = np.asarray(Wk); Wv = np.asarray(Wv)
    Wo = np.asarray(Wo)
    bq = np.asarray(bq); bk = np.asarray(bk); bv = np.asarray(bv)
    bo = np.asarray(bo)

    nc = _get_nc()
    in_maps = make_in_maps(query, key, value, mask, rel_pos_bias,
                           Wq, bq, Wk, bk, Wv, bv, Wo, bo)
    kw = _run_kwargs or {}
    res = run_bass_kernel_spmd(nc, in_maps, core_ids=list(range(N_CORES)), **kw)
    out = assemble_output(res.results, bv, Wo, bo)
    if _run_kwargs is not None:
        kernel._last_results = res
    return out

